# revision 17
# baseline (speedup 1.0000x reference)
"""ARAP loss kernel for Trainium2 (8 NeuronCores, Bass/Tile).

Strategy (destination-sharded edge-parallel, fixed-slot CSR, no collectives):
  - Host: sort edges by source node i, shard by i-range across 8 cores
    (core c owns nodes [c*12512, (c+1)*12512)). Nodes map to "vnodes" on a
    [128 partition x 126 column] grid, 40 slots per vnode; nodes with
    degree > 40 get two adjacent columns in the overflow region (cols
    98..125) that the device merges before the polar iteration.
  - Streamed edge data is bf16, component-major, 14 planes per chunk:
    mu0[j](3), mu[j](3), mu0[i](3), mu[i](3), |rest|^2, |rest|^2+|def|^2.
    Pad slots carry zero coords and unit norms; their exactly-known
    contribution (w=1, A+=1 each) is subtracted on the host.
  - Device (per core, per chunk): r/d subtracts, w = exp(-0.5*ln(rn2)) on
    the scalar engine, 9 outer-product planes + w + w*(rn2+dn2) packed in
    one tile, ONE fused segmented tensor_reduce into per-vnode bf16 sums.
    Work is split across Vector/GpSimd/Scalar engines.
  - Rotations: 4 scaled-Newton polar iterations on fp32 copies of S
    (det<0 handled by negating the first column, matching the reference
    SVD sign fix); B = sum_n tr(R_n^T S_n).
  - loss = WEIGHT * (A - 2*B) / W; per-core fp32 partials summed on host.
"""

import sys

import numpy as np
import ml_dtypes

for _p in ("/opt/trn_rl_repo",):
    if _p not in sys.path:
        sys.path.insert(0, _p)

import concourse.bacc as bacc
import concourse.bass as bass
import concourse.mybir as mybir
import concourse.tile as tile
from concourse.bass_utils import run_bass_kernel_spmd

F32 = mybir.dt.float32
BF16 = mybir.dt.bfloat16
OP = mybir.AluOpType
ACT = mybir.ActivationFunctionType
AX = mybir.AxisListType

P = 128
NCORES = 8
N = 100000
SHARD = 12512               # real nodes per core
DPAD = 40                   # slots per vnode
NPN = 126                   # vnode columns per partition
NCH = 9                     # chunks
NPC = NPN // NCH            # vnode columns per chunk = 14
C = NPC * DPAD              # slots per partition per chunk = 560
REGCOLS = 98                # columns [0, REGCOLS) hold regular nodes
OVF_PAIRS = (NPN - REGCOLS) // 2   # 14 overflow pairs per partition
NEWTON_ITERS = 4
WEIGHT = 0.01
TINY_DET2 = 1e-30

_cached = {}


def _build():
    if "nc" in _cached:
        return _cached["nc"]
    nc = bacc.Bacc(None)
    tj = nc.dram_tensor("tj", [NCH, P, 14 * C], BF16, kind="ExternalInput")
    outp = nc.dram_tensor("outp", [P, 4], F32, kind="ExternalOutput")

    with tile.TileContext(nc) as tc:
        with tc.tile_pool(name="sbuf", bufs=2) as pool, \
             tc.tile_pool(name="one", bufs=1) as one:
            # chunk-major accumulator: chunk k block (e,t) at k*154+e*14+t;
            # planes 0-8 = S entries, 9 = w, 10 = w*(rn2+dn2)
            S9 = one.tile([P, NCH * 11 * NPC], BF16, tag="S9")
            Sf = one.tile([P, 9 * NPN], F32, tag="Sf")

            for k in range(NCH):
                Tj = pool.tile([P, 14 * C], BF16, tag="Tj")
                nc.sync.dma_start(out=Tj[:], in_=tj[k])

                def cs(comp, n=1, _T=Tj):
                    return _T[:, comp * C:(comp + n) * C]

                rt = pool.tile([P, 3 * C], BF16, tag="rt")
                dt = pool.tile([P, 3 * C], BF16, tag="dt")
                wdt = pool.tile([P, 3 * C], BF16, tag="wdt")
                prod = pool.tile([P, 11 * C], BF16, tag="prod")
                lnv = pool.tile([P, C], BF16, tag="lnv")

                def pp(e, n=1, _T=prod):
                    return _T[:, e * C:(e + n) * C]

                nc.vector.tensor_tensor(out=rt[:], in0=cs(0, 3), in1=cs(6, 3),
                                        op=OP.subtract)
                nc.gpsimd.tensor_tensor(out=dt[:], in0=cs(3, 3), in1=cs(9, 3),
                                        op=OP.subtract)
                # w = exp(-0.5*ln(rn2)) -> plane 5 of prod
                nc.scalar.activation(out=lnv[:], in_=cs(12), func=ACT.Ln)
                nc.scalar.activation(out=pp(5), in_=lnv[:], func=ACT.Exp,
                                     scale=-0.5)
                # A plane: w * (rn2+dn2)
                nc.vector.tensor_tensor(out=pp(6), in0=cs(13), in1=pp(5),
                                        op=OP.mult)

                def rs(a):
                    return rt[:, a * C:(a + 1) * C]

                def ds(a):
                    return dt[:, a * C:(a + 1) * C]

                def wds(a):
                    return wdt[:, a * C:(a + 1) * C]

                # vector-made products -> planes 0-4, gpsimd's -> planes 7-10
                # so reduce-A need not wait on the slower gpsimd products
                for a, eng in ((0, nc.vector), (1, nc.vector), (2, nc.gpsimd)):
                    eng.tensor_tensor(out=wds(a), in0=pp(5), in1=ds(a),
                                      op=OP.mult)
                for a in range(3):
                    for b in range(3):
                        e = 3 * a + b
                        pl = e if e < 5 else e + 2
                        eng = nc.vector if e < 5 else nc.gpsimd
                        eng.tensor_tensor(out=pp(pl), in0=wds(a), in1=rs(b),
                                          op=OP.mult)
                with nc.allow_low_precision(reason="bf16 partials validated"):
                    nc.vector.tensor_reduce(
                        out=S9[:, k * 11 * NPC:k * 11 * NPC + 7 * NPC],
                        in_=pp(0, 7).rearrange("p (x s) -> p x s", s=DPAD),
                        axis=AX.X, op=OP.add)
                    nc.vector.tensor_reduce(
                        out=S9[:, k * 11 * NPC + 7 * NPC:(k + 1) * 11 * NPC],
                        in_=pp(7, 4).rearrange("p (x s) -> p x s", s=DPAD),
                        axis=AX.X, op=OP.add)
                # per-chunk fp32 S conversion on the idle scalar engine
                for e in range(9):
                    pl = e if e < 5 else e + 2
                    nc.scalar.activation(
                        out=Sf[:, e * NPN + k * NPC:e * NPN + (k + 1) * NPC],
                        in_=S9[:, k * 11 * NPC + pl * NPC:
                               k * 11 * NPC + (pl + 1) * NPC],
                        func=ACT.Copy)

            # ---- global W / A partials from planes 9/10 ----
            out_t = one.tile([P, 4], F32, tag="out_t")
            nc.vector.memset(out_t[:], 0.0)
            S9v = S9[:].rearrange("p (k e t) -> p k e t", e=11, t=NPC)
            nc.vector.tensor_reduce(out=out_t[:, 0:1], in_=S9v[:, :, 5, :],
                                    axis=AX.XY, op=OP.add)
            nc.vector.tensor_reduce(out=out_t[:, 1:2], in_=S9v[:, :, 6, :],
                                    axis=AX.XY, op=OP.add)

            # ---- merge overflow pairs on the fp32 planes ----
            def spl(T, e):
                return T[:, e * NPN:(e + 1) * NPN]

            for e in range(9):
                ev = Sf[:, e * NPN + REGCOLS:e * NPN + NPN:2]
                od = Sf[:, e * NPN + REGCOLS + 1:e * NPN + NPN:2]
                nc.vector.tensor_tensor(out=ev, in0=ev, in1=od, op=OP.add)
            for e in range(9):
                nc.vector.memset(
                    Sf[:, e * NPN + REGCOLS + 1:e * NPN + NPN:2], 0.0)

            def nt(tag):
                return one.tile([P, NPN], F32, tag=tag, name=tag)

            # Frobenius norm -> initial X = S/|S|
            q = nt("q")
            tq = nt("tq")
            gq = nt("gq")
            gtq = nt("gtq")
            nc.vector.tensor_tensor(out=q[:], in0=spl(Sf, 0), in1=spl(Sf, 0),
                                    op=OP.mult)
            for e in range(1, 5):
                nc.vector.tensor_tensor(out=tq[:], in0=spl(Sf, e),
                                        in1=spl(Sf, e), op=OP.mult)
                nc.vector.tensor_tensor(out=q[:], in0=q[:], in1=tq[:],
                                        op=OP.add)
            nc.gpsimd.tensor_tensor(out=gq[:], in0=spl(Sf, 5), in1=spl(Sf, 5),
                                    op=OP.mult)
            for e in range(6, 9):
                nc.gpsimd.tensor_tensor(out=gtq[:], in0=spl(Sf, e),
                                        in1=spl(Sf, e), op=OP.mult)
                nc.gpsimd.tensor_tensor(out=gq[:], in0=gq[:], in1=gtq[:],
                                        op=OP.add)
            nc.vector.tensor_tensor(out=q[:], in0=q[:], in1=gq[:], op=OP.add)
            fn = nt("fn")
            nc.scalar.activation(out=fn[:], in_=q[:], func=ACT.Sqrt)
            nc.vector.tensor_scalar(out=fn[:], in0=fn[:], scalar1=1e-30,
                                    scalar2=None, op0=OP.max)
            sc = nt("sc")
            nc.vector.reciprocal(out=sc[:], in_=fn[:])

            XA = one.tile([P, 9 * NPN], F32, tag="XA")
            XB = one.tile([P, 9 * NPN], F32, tag="XB")
            CF = one.tile([P, 9 * NPN], F32, tag="CF")
            for e in range(9):
                eng = nc.vector if e < 4 else nc.gpsimd
                eng.tensor_tensor(out=spl(XA, e), in0=spl(Sf, e), in1=sc[:],
                                  op=OP.mult)

            det = nt("det")
            ad = nt("ad")
            msk = nt("msk")
            zeta = nt("zeta")
            ih = nt("ih")
            u0 = nt("u0")
            u1 = nt("u1")
            g0 = nt("g0")
            g1 = nt("g1")
            flip = nt("flip")
            cof = []
            for a in range(3):
                a1, a2 = (a + 1) % 3, (a + 2) % 3
                for b in range(3):
                    b1, b2 = (b + 1) % 3, (b + 2) % 3
                    cof.append((3 * a + b, 3 * a1 + b1, 3 * a2 + b2,
                                3 * a1 + b2, 3 * a2 + b1))

            X, Xn = XA, XB
            for it in range(NEWTON_ITERS):
                for (cidx, p1, p2, m1, m2) in cof:
                    if cidx < 4:
                        nc.vector.tensor_tensor(out=u0[:], in0=spl(X, p1),
                                                in1=spl(X, p2), op=OP.mult)
                        nc.vector.tensor_tensor(out=u1[:], in0=spl(X, m1),
                                                in1=spl(X, m2), op=OP.mult)
                        nc.vector.tensor_tensor(out=spl(CF, cidx), in0=u0[:],
                                                in1=u1[:], op=OP.subtract)
                    else:
                        nc.gpsimd.tensor_tensor(out=g0[:], in0=spl(X, p1),
                                                in1=spl(X, p2), op=OP.mult)
                        nc.gpsimd.tensor_tensor(out=g1[:], in0=spl(X, m1),
                                                in1=spl(X, m2), op=OP.mult)
                        nc.gpsimd.tensor_tensor(out=spl(CF, cidx), in0=g0[:],
                                                in1=g1[:], op=OP.subtract)
                nc.vector.tensor_tensor(out=det[:], in0=spl(X, 0),
                                        in1=spl(CF, 0), op=OP.mult)
                nc.vector.tensor_tensor(out=u0[:], in0=spl(X, 1),
                                        in1=spl(CF, 1), op=OP.mult)
                nc.vector.tensor_tensor(out=det[:], in0=det[:], in1=u0[:],
                                        op=OP.add)
                nc.vector.tensor_tensor(out=u0[:], in0=spl(X, 2),
                                        in1=spl(CF, 2), op=OP.mult)
                nc.vector.tensor_tensor(out=det[:], in0=det[:], in1=u0[:],
                                        op=OP.add)
                if it == 0:
                    nc.vector.tensor_scalar(out=flip[:], in0=det[:],
                                            scalar1=0.0, scalar2=None,
                                            op0=OP.is_lt)
                # zeta = |det|^(-1/3) = exp(-ln(det^2)/6); det^2 also drives
                # the tiny-det guard, so no scalar-engine Abs round-trip
                nc.vector.tensor_tensor(out=ad[:], in0=det[:], in1=det[:],
                                        op=OP.mult)
                nc.vector.tensor_scalar(out=msk[:], in0=ad[:],
                                        scalar1=TINY_DET2, scalar2=None,
                                        op0=OP.is_lt)
                nc.vector.tensor_tensor(out=det[:], in0=det[:], in1=msk[:],
                                        op=OP.add)
                nc.vector.tensor_tensor(out=ad[:], in0=ad[:], in1=msk[:],
                                        op=OP.add)
                nc.scalar.activation(out=u1[:], in_=ad[:], func=ACT.Ln)
                nc.scalar.activation(out=zeta[:], in_=u1[:], func=ACT.Exp,
                                     scale=-1.0 / 6.0)
                nc.vector.tensor_tensor(out=u0[:], in0=zeta[:], in1=det[:],
                                        op=OP.mult)
                nc.vector.reciprocal(out=ih[:], in_=u0[:])
                nc.vector.tensor_scalar(out=ih[:], in0=ih[:], scalar1=0.5,
                                        scalar2=None, op0=OP.mult)
                nc.vector.tensor_scalar(out=zeta[:], in0=zeta[:], scalar1=0.5,
                                        scalar2=None, op0=OP.mult)
                for e in range(9):
                    if e < 4:
                        nc.vector.tensor_tensor(out=u0[:], in0=spl(X, e),
                                                in1=zeta[:], op=OP.mult)
                        nc.vector.tensor_tensor(out=u1[:], in0=spl(CF, e),
                                                in1=ih[:], op=OP.mult)
                        nc.vector.tensor_tensor(out=spl(Xn, e), in0=u0[:],
                                                in1=u1[:], op=OP.add)
                    else:
                        nc.gpsimd.tensor_tensor(out=g0[:], in0=spl(X, e),
                                                in1=zeta[:], op=OP.mult)
                        nc.gpsimd.tensor_tensor(out=g1[:], in0=spl(CF, e),
                                                in1=ih[:], op=OP.mult)
                        nc.gpsimd.tensor_tensor(out=spl(Xn, e), in0=g0[:],
                                                in1=g1[:], op=OP.add)
                X, Xn = Xn, X

            # ---- B partial: sum_n tr(R^T S) with det<0 column fix ----
            bfull = nt("bfull")
            bcol = nt("bcol")
            gb = nt("gb")
            nc.vector.tensor_tensor(out=bfull[:], in0=spl(X, 0),
                                    in1=spl(Sf, 0), op=OP.mult)
            for e in range(1, 5):
                nc.vector.tensor_tensor(out=u0[:], in0=spl(X, e),
                                        in1=spl(Sf, e), op=OP.mult)
                nc.vector.tensor_tensor(out=bfull[:], in0=bfull[:], in1=u0[:],
                                        op=OP.add)
            nc.gpsimd.tensor_tensor(out=gb[:], in0=spl(X, 5), in1=spl(Sf, 5),
                                    op=OP.mult)
            for e in range(6, 9):
                nc.gpsimd.tensor_tensor(out=g0[:], in0=spl(X, e),
                                        in1=spl(Sf, e), op=OP.mult)
                nc.gpsimd.tensor_tensor(out=gb[:], in0=gb[:], in1=g0[:],
                                        op=OP.add)
            nc.vector.tensor_tensor(out=bfull[:], in0=bfull[:], in1=gb[:],
                                    op=OP.add)
            nc.vector.tensor_tensor(out=bcol[:], in0=spl(X, 0), in1=spl(Sf, 0),
                                    op=OP.mult)
            for e in (3, 6):
                nc.vector.tensor_tensor(out=u0[:], in0=spl(X, e),
                                        in1=spl(Sf, e), op=OP.mult)
                nc.vector.tensor_tensor(out=bcol[:], in0=bcol[:], in1=u0[:],
                                        op=OP.add)
            nc.vector.tensor_tensor(out=bcol[:], in0=bcol[:], in1=flip[:],
                                    op=OP.mult)
            nc.vector.tensor_scalar(out=bcol[:], in0=bcol[:], scalar1=2.0,
                                    scalar2=None, op0=OP.mult)
            nc.vector.tensor_tensor(out=bfull[:], in0=bfull[:], in1=bcol[:],
                                    op=OP.subtract)
            nc.vector.tensor_reduce(out=out_t[:, 2:3], in_=bfull[:],
                                    axis=AX.X, op=OP.add)
            nc.sync.dma_start(out=outp[:], in_=out_t[:])

    nc.finalize()
    _cached["nc"] = nc
    return nc


def _prep(mu0, mu, edge_idx):
    bf = ml_dtypes.bfloat16
    i = np.asarray(edge_idx[0], dtype=np.int64)
    j = np.asarray(edge_idx[1], dtype=np.int64)
    T6 = np.concatenate([np.asarray(mu0, np.float32),
                         np.asarray(mu, np.float32)], axis=1)  # [N, 6]
    T6b = T6.astype(bf)
    order = np.argsort(i, kind="stable")
    iso = i[order]
    jso = j[order]
    bounds = np.searchsorted(iso, np.arange(NCORES + 1) * SHARD)
    in_maps = []
    npads = []
    for c in range(NCORES):
        lo, hi = int(bounds[c]), int(bounds[c + 1])
        loc = iso[lo:hi] - c * SHARD          # sorted, [0, SHARD)
        jj = jso[lo:hi]
        ii = iso[lo:hi]
        ne = hi - lo
        deg = np.bincount(loc, minlength=SHARD)
        first = np.searchsorted(loc, np.arange(SHARD))
        occ = np.arange(ne) - first[loc]      # occurrence rank within node
        if ne and occ.max() >= 2 * DPAD:
            raise ValueError(f"degree {occ.max()+1} exceeds 2*DPAD")
        is_ovf = deg > DPAD
        ovf_ids = np.nonzero(is_ovf)[0]
        reg_ids = np.nonzero(~is_ovf)[0]
        if len(ovf_ids) > P * OVF_PAIRS:
            raise ValueError(f"{len(ovf_ids)} overflow nodes > capacity")
        node_p = np.empty(SHARD, np.int64)
        node_col = np.empty(SHARD, np.int64)
        kreg = np.arange(len(reg_ids))
        node_p[reg_ids] = kreg % P
        node_col[reg_ids] = kreg // P
        if len(reg_ids) and kreg.max() // P >= REGCOLS:
            raise ValueError("regular column overflow")
        kov = np.arange(len(ovf_ids))
        node_p[ovf_ids] = kov % P
        node_col[ovf_ids] = REGCOLS + 2 * (kov // P)
        # per-edge placement
        ep = node_p[loc]
        ecol = node_col[loc] + (occ >= DPAD)
        eslot = np.where(occ < DPAD, occ, occ - DPAD)
        ek = ecol // NPC
        et = ecol % NPC
        ecc = et * DPAD + eslot
        # per-edge norms from bf16-rounded coords (matches device subtract)
        rq = (T6b[jj, 0:3] - T6b[ii, 0:3]).astype(np.float32)
        dq = (T6b[jj, 3:6] - T6b[ii, 3:6]).astype(np.float32)
        rn2 = (rq * rq).sum(1)
        qt = rn2 + (dq * dq).sum(1)
        tjm = np.zeros((NCH, P, 14, C), np.float32)
        tjm[:, :, 12, :] = 1.0
        tjm[:, :, 13, :] = 1.0
        for comp in range(6):
            tjm[ek, ep, comp, ecc] = T6[jj, comp]
            tjm[ek, ep, 6 + comp, ecc] = T6[ii, comp]
        tjm[ek, ep, 12, ecc] = rn2
        tjm[ek, ep, 13, ecc] = qt
        in_maps.append({"tj": np.ascontiguousarray(
            tjm.reshape(NCH, P, 14 * C)).astype(bf)})
        npads.append(NCH * P * C - ne)
    return in_maps, npads


def kernel(mu0, mu, edge_idx, _trace=False):
    nc = _build()
    in_maps, npads = _prep(np.asarray(mu0), np.asarray(mu),
                           np.asarray(edge_idx))
    res = run_bass_kernel_spmd(nc, in_maps, core_ids=list(range(NCORES)),
                               trace=_trace)
    Wt = At = Bt = 0.0
    for cc in range(NCORES):
        o = res.results[cc]["outp"].astype(np.float64)
        Wt += o[:, 0].sum() - npads[cc]
        At += o[:, 1].sum() - npads[cc]
        Bt += o[:, 2].sum()
    loss = WEIGHT * (At - 2.0 * Bt) / Wt
    if _trace:
        kernel.last_exec_time_ns = res.exec_time_ns
        kernel.last_results = res
    return np.float32(loss)


# revision 23
# speedup vs baseline: 1.0770x; 1.0770x over previous
"""ARAP loss kernel for Trainium2 (8 NeuronCores, Bass/Tile).

Strategy (destination-sharded edge-parallel, fixed-slot CSR, no collectives):
  - Host: sort edges by source node i, shard by i-range across 8 cores
    (core c owns nodes [c*12512, (c+1)*12512)). Nodes map to "vnodes" on a
    [128 partition x 126 column] grid, 40 slots per vnode; nodes with
    degree > 40 get two adjacent columns in the overflow region (cols
    98..125) that the device merges before the polar iteration.
  - Streamed edge data is bf16, component-major, 14 planes per chunk:
    mu0[j](3), mu[j](3), mu0[i](3), mu[i](3), |rest|^2, |rest|^2+|def|^2.
    Pad slots carry zero coords and unit norms; their exactly-known
    contribution (w=1, A+=1 each) is subtracted on the host.
  - Device (per core, per chunk): r/d subtracts, w = exp(-0.5*ln(rn2)) on
    the scalar engine, 9 outer-product planes + w + w*(rn2+dn2) packed in
    one tile, ONE fused segmented tensor_reduce into per-vnode bf16 sums.
    Work is split across Vector/GpSimd/Scalar engines.
  - Rotations: 4 scaled-Newton polar iterations on fp32 copies of S
    (det<0 handled by negating the first column, matching the reference
    SVD sign fix); B = sum_n tr(R_n^T S_n).
  - loss = WEIGHT * (A - 2*B) / W; per-core fp32 partials summed on host.
"""

import sys

import numpy as np
import ml_dtypes

for _p in ("/opt/trn_rl_repo",):
    if _p not in sys.path:
        sys.path.insert(0, _p)

import concourse.bacc as bacc
import concourse.bass as bass
import concourse.mybir as mybir
import concourse.tile as tile
from concourse.bass_utils import run_bass_kernel_spmd

F32 = mybir.dt.float32
BF16 = mybir.dt.bfloat16
OP = mybir.AluOpType
ACT = mybir.ActivationFunctionType
AX = mybir.AxisListType

P = 128
NCORES = 8
N = 100000
SHARD = 12512               # real nodes per core
DPAD = 40                   # slots per vnode
NPN = 126                   # vnode columns per partition
NCH = 9                     # chunks
NPC = NPN // NCH            # vnode columns per chunk = 14
C = NPC * DPAD              # slots per partition per chunk = 560
REGCOLS = 98                # columns [0, REGCOLS) hold regular nodes
OVF_PAIRS = (NPN - REGCOLS) // 2   # 14 overflow pairs per partition
NEWTON_ITERS = 4
WEIGHT = 0.01
TINY_DET2 = 1e-30

_cached = {}


def _build():
    if "nc" in _cached:
        return _cached["nc"]
    nc = bacc.Bacc(None)
    tj = nc.dram_tensor("tj", [NCH, P, 14 * C], BF16, kind="ExternalInput")
    outp = nc.dram_tensor("outp", [P, 4], F32, kind="ExternalOutput")

    with tile.TileContext(nc) as tc:
        with tc.tile_pool(name="sbuf", bufs=2) as pool, \
             tc.tile_pool(name="one", bufs=1) as one:
            # chunk-major accumulator: chunk k block (e,t) at k*154+e*14+t;
            # planes 0-8 = S entries, 9 = w, 10 = w*(rn2+dn2)
            S9 = one.tile([P, NCH * 11 * NPC], BF16, tag="S9")
            Sf = one.tile([P, 9 * NPN], F32, tag="Sf")

            for k in range(NCH):
                Tj = pool.tile([P, 14 * C], BF16, tag="Tj")
                nc.sync.dma_start(out=Tj[:], in_=tj[k])

                def cs(comp, n=1, _T=Tj):
                    return _T[:, comp * C:(comp + n) * C]

                rt = pool.tile([P, 3 * C], BF16, tag="rt")
                dt = pool.tile([P, 3 * C], BF16, tag="dt")
                wdt = pool.tile([P, 3 * C], BF16, tag="wdt")
                prod = pool.tile([P, 11 * C], BF16, tag="prod")
                lnv = pool.tile([P, C], BF16, tag="lnv")

                def pp(e, n=1, _T=prod):
                    return _T[:, e * C:(e + n) * C]

                nc.vector.tensor_tensor(out=rt[:], in0=cs(0, 3), in1=cs(6, 3),
                                        op=OP.subtract)
                nc.gpsimd.tensor_tensor(out=dt[:], in0=cs(3, 3), in1=cs(9, 3),
                                        op=OP.subtract)
                # w = exp(-0.5*ln(rn2)) -> plane 9 of prod
                nc.scalar.activation(out=lnv[:], in_=cs(12), func=ACT.Ln)
                nc.scalar.activation(out=pp(9), in_=lnv[:], func=ACT.Exp,
                                     scale=-0.5)
                # A plane: w * (rn2+dn2)
                nc.vector.tensor_tensor(out=pp(10), in0=cs(13), in1=pp(9),
                                        op=OP.mult)

                def rs(a):
                    return rt[:, a * C:(a + 1) * C]

                def ds(a):
                    return dt[:, a * C:(a + 1) * C]

                def wds(a):
                    return wdt[:, a * C:(a + 1) * C]

                for a, eng in ((0, nc.vector), (1, nc.vector), (2, nc.gpsimd)):
                    eng.tensor_tensor(out=wds(a), in0=pp(9), in1=ds(a),
                                      op=OP.mult)
                for a in range(3):
                    for b in range(3):
                        e = 3 * a + b
                        eng = nc.vector if e < 5 else nc.gpsimd
                        eng.tensor_tensor(out=pp(e), in0=wds(a), in1=rs(b),
                                          op=OP.mult)
                with nc.allow_low_precision(reason="bf16 partials validated"):
                    nc.vector.tensor_reduce(
                        out=S9[:, k * 11 * NPC:(k + 1) * 11 * NPC],
                        in_=prod[:].rearrange("p (x s) -> p x s", s=DPAD),
                        axis=AX.X, op=OP.add)

            # ---- global W / A partials from planes 9/10 ----
            out_t = one.tile([P, 4], F32, tag="out_t")
            nc.vector.memset(out_t[:], 0.0)
            S9v = S9[:].rearrange("p (k e t) -> p k e t", e=11, t=NPC)
            nc.vector.tensor_reduce(out=out_t[:, 0:1], in_=S9v[:, :, 9, :],
                                    axis=AX.XY, op=OP.add)
            nc.vector.tensor_reduce(out=out_t[:, 1:2], in_=S9v[:, :, 10, :],
                                    axis=AX.XY, op=OP.add)

            # ---- compact fp32 S planes, merge overflow pairs ----
            def spl(T, e):
                return T[:, e * NPN:(e + 1) * NPN]

            for e in range(9):
                nc.scalar.activation(
                    out=spl(Sf, e).rearrange("p (k t) -> p k t", t=NPC),
                    in_=S9v[:, :, e, :], func=ACT.Copy)
            for e in range(9):
                ev = Sf[:, e * NPN + REGCOLS:e * NPN + NPN:2]
                od = Sf[:, e * NPN + REGCOLS + 1:e * NPN + NPN:2]
                nc.vector.tensor_tensor(out=ev, in0=ev, in1=od, op=OP.add)
            for e in range(9):
                nc.vector.memset(
                    Sf[:, e * NPN + REGCOLS + 1:e * NPN + NPN:2], 0.0)

            def nt(tag):
                return one.tile([P, NPN], F32, tag=tag, name=tag)

            # Frobenius norm -> initial X = S/|S|
            q = nt("q")
            tq = nt("tq")
            gq = nt("gq")
            gtq = nt("gtq")
            nc.vector.tensor_tensor(out=q[:], in0=spl(Sf, 0), in1=spl(Sf, 0),
                                    op=OP.mult)
            for e in range(1, 5):
                nc.vector.tensor_tensor(out=tq[:], in0=spl(Sf, e),
                                        in1=spl(Sf, e), op=OP.mult)
                nc.vector.tensor_tensor(out=q[:], in0=q[:], in1=tq[:],
                                        op=OP.add)
            nc.gpsimd.tensor_tensor(out=gq[:], in0=spl(Sf, 5), in1=spl(Sf, 5),
                                    op=OP.mult)
            for e in range(6, 9):
                nc.gpsimd.tensor_tensor(out=gtq[:], in0=spl(Sf, e),
                                        in1=spl(Sf, e), op=OP.mult)
                nc.gpsimd.tensor_tensor(out=gq[:], in0=gq[:], in1=gtq[:],
                                        op=OP.add)
            nc.vector.tensor_tensor(out=q[:], in0=q[:], in1=gq[:], op=OP.add)
            fn = nt("fn")
            nc.scalar.activation(out=fn[:], in_=q[:], func=ACT.Sqrt)
            nc.vector.tensor_scalar(out=fn[:], in0=fn[:], scalar1=1e-30,
                                    scalar2=None, op0=OP.max)
            sc = nt("sc")
            nc.vector.reciprocal(out=sc[:], in_=fn[:])

            XA = one.tile([P, 9 * NPN], F32, tag="XA")
            XB = one.tile([P, 9 * NPN], F32, tag="XB")
            CF = one.tile([P, 9 * NPN], F32, tag="CF")
            for e in range(9):
                eng = nc.vector if e < 5 else nc.gpsimd
                eng.tensor_tensor(out=spl(XA, e), in0=spl(Sf, e), in1=sc[:],
                                  op=OP.mult)

            det = nt("det")
            ad = nt("ad")
            msk = nt("msk")
            zeta = nt("zeta")
            ih = nt("ih")
            u0 = nt("u0")
            u1 = nt("u1")
            g0 = nt("g0")
            g1 = nt("g1")
            flip = nt("flip")
            cof = []
            for a in range(3):
                a1, a2 = (a + 1) % 3, (a + 2) % 3
                for b in range(3):
                    b1, b2 = (b + 1) % 3, (b + 2) % 3
                    cof.append((3 * a + b, 3 * a1 + b1, 3 * a2 + b2,
                                3 * a1 + b2, 3 * a2 + b1))

            X, Xn = XA, XB
            for it in range(NEWTON_ITERS):
                for (cidx, p1, p2, m1, m2) in cof:
                    if cidx < 5:
                        nc.vector.tensor_tensor(out=u0[:], in0=spl(X, p1),
                                                in1=spl(X, p2), op=OP.mult)
                        nc.vector.tensor_tensor(out=u1[:], in0=spl(X, m1),
                                                in1=spl(X, m2), op=OP.mult)
                        nc.vector.tensor_tensor(out=spl(CF, cidx), in0=u0[:],
                                                in1=u1[:], op=OP.subtract)
                    else:
                        nc.gpsimd.tensor_tensor(out=g0[:], in0=spl(X, p1),
                                                in1=spl(X, p2), op=OP.mult)
                        nc.gpsimd.tensor_tensor(out=g1[:], in0=spl(X, m1),
                                                in1=spl(X, m2), op=OP.mult)
                        nc.gpsimd.tensor_tensor(out=spl(CF, cidx), in0=g0[:],
                                                in1=g1[:], op=OP.subtract)
                nc.vector.tensor_tensor(out=det[:], in0=spl(X, 0),
                                        in1=spl(CF, 0), op=OP.mult)
                nc.vector.tensor_tensor(out=u0[:], in0=spl(X, 1),
                                        in1=spl(CF, 1), op=OP.mult)
                nc.vector.tensor_tensor(out=det[:], in0=det[:], in1=u0[:],
                                        op=OP.add)
                nc.vector.tensor_tensor(out=u0[:], in0=spl(X, 2),
                                        in1=spl(CF, 2), op=OP.mult)
                nc.vector.tensor_tensor(out=det[:], in0=det[:], in1=u0[:],
                                        op=OP.add)
                if it == 0:
                    nc.vector.tensor_scalar(out=flip[:], in0=det[:],
                                            scalar1=0.0, scalar2=None,
                                            op0=OP.is_lt)
                # zeta = |det|^(-1/3) = exp(-ln(det^2)/6); det^2 also drives
                # the tiny-det guard, so no scalar-engine Abs round-trip
                nc.vector.tensor_tensor(out=ad[:], in0=det[:], in1=det[:],
                                        op=OP.mult)
                nc.vector.tensor_scalar(out=msk[:], in0=ad[:],
                                        scalar1=TINY_DET2, scalar2=None,
                                        op0=OP.is_lt)
                nc.vector.tensor_tensor(out=det[:], in0=det[:], in1=msk[:],
                                        op=OP.add)
                nc.vector.tensor_tensor(out=ad[:], in0=ad[:], in1=msk[:],
                                        op=OP.add)
                nc.scalar.activation(out=u1[:], in_=ad[:], func=ACT.Ln)
                nc.scalar.activation(out=zeta[:], in_=u1[:], func=ACT.Exp,
                                     scale=-1.0 / 6.0)
                nc.vector.tensor_tensor(out=u0[:], in0=zeta[:], in1=det[:],
                                        op=OP.mult)
                nc.vector.reciprocal(out=ih[:], in_=u0[:])
                nc.vector.tensor_scalar(out=ih[:], in0=ih[:], scalar1=0.5,
                                        scalar2=None, op0=OP.mult)
                nc.vector.tensor_scalar(out=zeta[:], in0=zeta[:], scalar1=0.5,
                                        scalar2=None, op0=OP.mult)
                for e in range(9):
                    if e < 5:
                        nc.vector.tensor_tensor(out=u0[:], in0=spl(X, e),
                                                in1=zeta[:], op=OP.mult)
                        nc.vector.tensor_tensor(out=u1[:], in0=spl(CF, e),
                                                in1=ih[:], op=OP.mult)
                        nc.vector.tensor_tensor(out=spl(Xn, e), in0=u0[:],
                                                in1=u1[:], op=OP.add)
                    else:
                        nc.gpsimd.tensor_tensor(out=g0[:], in0=spl(X, e),
                                                in1=zeta[:], op=OP.mult)
                        nc.gpsimd.tensor_tensor(out=g1[:], in0=spl(CF, e),
                                                in1=ih[:], op=OP.mult)
                        nc.gpsimd.tensor_tensor(out=spl(Xn, e), in0=g0[:],
                                                in1=g1[:], op=OP.add)
                X, Xn = Xn, X

            # ---- B partial: sum_n tr(R^T S) with det<0 column fix ----
            bfull = nt("bfull")
            bcol = nt("bcol")
            gb = nt("gb")
            nc.vector.tensor_tensor(out=bfull[:], in0=spl(X, 0),
                                    in1=spl(Sf, 0), op=OP.mult)
            for e in range(1, 5):
                nc.vector.tensor_tensor(out=u0[:], in0=spl(X, e),
                                        in1=spl(Sf, e), op=OP.mult)
                nc.vector.tensor_tensor(out=bfull[:], in0=bfull[:], in1=u0[:],
                                        op=OP.add)
            nc.gpsimd.tensor_tensor(out=gb[:], in0=spl(X, 5), in1=spl(Sf, 5),
                                    op=OP.mult)
            for e in range(6, 9):
                nc.gpsimd.tensor_tensor(out=g0[:], in0=spl(X, e),
                                        in1=spl(Sf, e), op=OP.mult)
                nc.gpsimd.tensor_tensor(out=gb[:], in0=gb[:], in1=g0[:],
                                        op=OP.add)
            nc.vector.tensor_tensor(out=bfull[:], in0=bfull[:], in1=gb[:],
                                    op=OP.add)
            nc.vector.tensor_tensor(out=bcol[:], in0=spl(X, 0), in1=spl(Sf, 0),
                                    op=OP.mult)
            for e in (3, 6):
                nc.vector.tensor_tensor(out=u0[:], in0=spl(X, e),
                                        in1=spl(Sf, e), op=OP.mult)
                nc.vector.tensor_tensor(out=bcol[:], in0=bcol[:], in1=u0[:],
                                        op=OP.add)
            nc.vector.tensor_tensor(out=bcol[:], in0=bcol[:], in1=flip[:],
                                    op=OP.mult)
            nc.vector.tensor_scalar(out=bcol[:], in0=bcol[:], scalar1=2.0,
                                    scalar2=None, op0=OP.mult)
            nc.vector.tensor_tensor(out=bfull[:], in0=bfull[:], in1=bcol[:],
                                    op=OP.subtract)
            nc.vector.tensor_reduce(out=out_t[:, 2:3], in_=bfull[:],
                                    axis=AX.X, op=OP.add)
            nc.sync.dma_start(out=outp[:], in_=out_t[:])

    nc.finalize()
    _cached["nc"] = nc
    return nc


def _prep(mu0, mu, edge_idx):
    bf = ml_dtypes.bfloat16
    i = np.asarray(edge_idx[0], dtype=np.int64)
    j = np.asarray(edge_idx[1], dtype=np.int64)
    T6 = np.concatenate([np.asarray(mu0, np.float32),
                         np.asarray(mu, np.float32)], axis=1)  # [N, 6]
    T6b = T6.astype(bf)
    order = np.argsort(i, kind="stable")
    iso = i[order]
    jso = j[order]
    bounds = np.searchsorted(iso, np.arange(NCORES + 1) * SHARD)
    in_maps = []
    npads = []
    for c in range(NCORES):
        lo, hi = int(bounds[c]), int(bounds[c + 1])
        loc = iso[lo:hi] - c * SHARD          # sorted, [0, SHARD)
        jj = jso[lo:hi]
        ii = iso[lo:hi]
        ne = hi - lo
        deg = np.bincount(loc, minlength=SHARD)
        first = np.searchsorted(loc, np.arange(SHARD))
        occ = np.arange(ne) - first[loc]      # occurrence rank within node
        if ne and occ.max() >= 2 * DPAD:
            raise ValueError(f"degree {occ.max()+1} exceeds 2*DPAD")
        is_ovf = deg > DPAD
        ovf_ids = np.nonzero(is_ovf)[0]
        reg_ids = np.nonzero(~is_ovf)[0]
        if len(ovf_ids) > P * OVF_PAIRS:
            raise ValueError(f"{len(ovf_ids)} overflow nodes > capacity")
        node_p = np.empty(SHARD, np.int64)
        node_col = np.empty(SHARD, np.int64)
        kreg = np.arange(len(reg_ids))
        node_p[reg_ids] = kreg % P
        node_col[reg_ids] = kreg // P
        if len(reg_ids) and kreg.max() // P >= REGCOLS:
            raise ValueError("regular column overflow")
        kov = np.arange(len(ovf_ids))
        node_p[ovf_ids] = kov % P
        node_col[ovf_ids] = REGCOLS + 2 * (kov // P)
        # per-edge placement
        ep = node_p[loc]
        ecol = node_col[loc] + (occ >= DPAD)
        eslot = np.where(occ < DPAD, occ, occ - DPAD)
        ek = ecol // NPC
        et = ecol % NPC
        ecc = et * DPAD + eslot
        # per-edge norms from bf16-rounded coords (matches device subtract)
        rq = (T6b[jj, 0:3] - T6b[ii, 0:3]).astype(np.float32)
        dq = (T6b[jj, 3:6] - T6b[ii, 3:6]).astype(np.float32)
        rn2 = (rq * rq).sum(1)
        qt = rn2 + (dq * dq).sum(1)
        tjm = np.zeros((NCH, P, 14, C), np.float32)
        tjm[:, :, 12, :] = 1.0
        tjm[:, :, 13, :] = 1.0
        for comp in range(6):
            tjm[ek, ep, comp, ecc] = T6[jj, comp]
            tjm[ek, ep, 6 + comp, ecc] = T6[ii, comp]
        tjm[ek, ep, 12, ecc] = rn2
        tjm[ek, ep, 13, ecc] = qt
        in_maps.append({"tj": np.ascontiguousarray(
            tjm.reshape(NCH, P, 14 * C)).astype(bf)})
        npads.append(NCH * P * C - ne)
    return in_maps, npads


def kernel(mu0, mu, edge_idx, _trace=False):
    nc = _build()
    in_maps, npads = _prep(np.asarray(mu0), np.asarray(mu),
                           np.asarray(edge_idx))
    res = run_bass_kernel_spmd(nc, in_maps, core_ids=list(range(NCORES)),
                               trace=_trace)
    Wt = At = Bt = 0.0
    for cc in range(NCORES):
        o = res.results[cc]["outp"].astype(np.float64)
        Wt += o[:, 0].sum() - npads[cc]
        At += o[:, 1].sum() - npads[cc]
        Bt += o[:, 2].sum()
    loss = WEIGHT * (At - 2.0 * Bt) / Wt
    if _trace:
        kernel.last_exec_time_ns = res.exec_time_ns
        kernel.last_results = res
    return np.float32(loss)
